# revision 4
# baseline (speedup 1.0000x reference)
"""Trainium2 Bass kernel for 2D erosion (3x3 sliding-window min) on
x: (8, 4, 1024, 1024) f32, borders padded with +1e9 (pad never wins).

Strategy: pure data parallel over the 32 (b, c) images -> 4 images per core.
Device compute runs in bf16 (harness gate is rel_err < 2e-2; bf16 rounding is
monotone so min commutes with it -> error <= 2^-9): halves DMA bytes and
doubles DVE throughput (2x_1p mode needs 2-byte dtype + unit-stride innermost
dims, which the op formulation below maintains).

Per-core DRAM input is a (4101, 1024) bf16 stack: 4 images with one 1e9 pad
row between/around. Per image, one overlapping load puts DRAM rows
8p-1 .. 8p+8 (10 rows, 20KB) in partition p — the +-1 halo rows ride along,
so the vertical pass needs no cross-partition traffic and no separate halo
DMA.

Vertical (H) pass, 3 DVE ops via pair-sharing (x rows indexed 0..9 in-tile,
v[r] = min over x[r .. r+2]):
    s[k]      = min(x[2k+1], x[2k+2])   k=0..3
    v[even r] = min(x[r], s[r/2])
    v[odd r]  = min(s[(r-1)/2], x[r+2])
All operands are 3D APs with unit-stride 1024-wide innermost dims -> 2x mode.

Horizontal (W) pass, shift formulation (unit strides, 2x mode), with the
final combine offloaded to GPSIMD to balance engine load:
    t[j] = min(v[j], v[j+1])            (DVE)
    o[j] = min(t[j-1], t[j])            (GPSIMD)
Row-boundary columns (first/last of each 1024-wide row) shrink to a 2-tap
window = the adjacent t value; one tiny strided copy per image fixes both
edges of all 8 rows.

Loads ride the SP HWDGE ring, stores the ACT ring, so they don't queue
behind each other.
"""

import numpy as np
import ml_dtypes

import concourse.bass as bass
import concourse.bacc as bacc
import concourse.mybir as mybir
from concourse.tile import TileContext
from concourse.bass_utils import run_bass_kernel_spmd

N_CORES = 8
B, C, H, W = 8, 4, 1024, 1024
IMGS = B * C // N_CORES  # images per core = 4
P = 128                  # SBUF partitions
R = H // P               # image rows per partition = 8
RL = R + 2               # loaded rows per partition (incl +-1 halo)
F = R * W                # free-dim elements per partition = 8192
PAD = 1.0e9
XROWS = IMGS * (H + 1) + 1  # padded per-core input rows = 4101
BF16 = mybir.dt.bfloat16
MIN = mybir.AluOpType.min
NP_BF16 = ml_dtypes.bfloat16

_NC_CACHE = {}


LOOP_BODY_REPS = 4  # reps unrolled inside the hardware loop body


def _emit_image(nc, pools, i):
    """Emit load -> H pass -> W pass -> store for image i."""
    xpool, spool, vpool, tpool, opool = pools
    x, y = nc._x, nc._y
    base = 1 + i * (H + 1)  # first image row in the padded stack

    xt = xpool.tile([P, RL * W], BF16)
    # overlapping load: partition p <- DRAM rows base-1+8p .. base+8+8p
    src = bass.AP(x, (base - 1) * W, [[R * W, P], [1, RL * W]])
    nc.sync.dma_start(out=xt, in_=src)

    xr = xt.rearrange("p (r w) -> p r w", r=RL)
    s = spool.tile([P, (R // 2) * W], BF16)
    sr = s.rearrange("p (r w) -> p r w", r=R // 2)
    v = vpool.tile([P, F], BF16)
    vr = v.rearrange("p (r w) -> p r w", r=R)

    # ---- H pass: v[r] = min(x[r], x[r+1], x[r+2]) (tile rows) ----
    nc.vector.tensor_tensor(
        out=sr, in0=xr[:, 1:9:2, :], in1=xr[:, 2:10:2, :], op=MIN
    )
    nc.vector.tensor_tensor(
        out=vr[:, 0:R:2, :], in0=xr[:, 0:R:2, :], in1=sr, op=MIN
    )
    nc.vector.tensor_tensor(
        out=vr[:, 1:R:2, :], in0=sr, in1=xr[:, 3:RL:2, :], op=MIN
    )

    # ---- W pass: o[j] = min(v[j-1], v[j], v[j+1]) within rows ----
    t = tpool.tile([P, F], BF16)  # t[0..F-2] valid
    nc.vector.tensor_tensor(
        out=t[:, 0 : F - 1], in0=v[:, 0 : F - 1], in1=v[:, 1:F], op=MIN
    )
    o = opool.tile([P, F], BF16)
    nc.vector.tensor_tensor(
        out=o[:, 1 : F - 1], in0=t[:, 0 : F - 2], in1=t[:, 1 : F - 1], op=MIN
    )
    # per-row first/last column: window shrinks to 2 taps = t value
    orr = o.rearrange("p (r w) -> p r w", r=R)
    tr = t.rearrange("p (r w) -> p r w", r=R)
    nc.vector.tensor_copy(
        out=orr[:, :, 0 : W : W - 1], in_=tr[:, :, 0 : W - 1 : W - 2]
    )

    # store on the ACT HWDGE ring (parallel to SP loads)
    ym = y[i].rearrange("(p r) w -> p (r w)", p=P)
    nc.scalar.dma_start(out=ym, in_=o)


def _build_nc(reps=1):
    nc = bacc.Bacc()
    nc._x = nc.dram_tensor("x", (XROWS, W), BF16, kind="ExternalInput")
    nc._y = nc.dram_tensor("y", (IMGS, H, W), BF16, kind="ExternalOutput")

    with TileContext(nc) as tc:
        with (
            tc.tile_pool(name="xp", bufs=3) as xpool,
            tc.tile_pool(name="sp", bufs=2) as spool,
            tc.tile_pool(name="vp", bufs=2) as vpool,
            tc.tile_pool(name="tp", bufs=2) as tpool,
            tc.tile_pool(name="op", bufs=2) as opool,
        ):
            pools = (xpool, spool, vpool, tpool, opool)
            if reps <= 48:
                for i in [im for _ in range(reps) for im in range(IMGS)]:
                    _emit_image(nc, pools, i)
            else:
                # timing mode: hardware loop keeps the NEFF compact so reps
                # can be large enough to swamp host/tunnel timing noise
                n_iter, rem = divmod(reps, LOOP_BODY_REPS)
                with tc.For_i(0, n_iter, 1):
                    for i in [
                        im for _ in range(LOOP_BODY_REPS) for im in range(IMGS)
                    ]:
                        _emit_image(nc, pools, i)
                for i in [im for _ in range(rem) for im in range(IMGS)]:
                    _emit_image(nc, pools, i)

    nc.finalize()
    return nc


def _get_nc(reps=1):
    if reps not in _NC_CACHE:
        _NC_CACHE[reps] = _build_nc(reps)
    return _NC_CACHE[reps]


def _to_bf16(x):
    """f32 -> bf16 with round-to-nearest-even (vectorized bit trick)."""
    u = np.ascontiguousarray(x, dtype=np.float32).view(np.uint32)
    r = ((u + 0x7FFF + ((u >> 16) & 1)) >> 16).astype(np.uint16)
    return r.view(NP_BF16)


def _pad_shard(shard_bf16):
    """(IMGS, H, W) bf16 -> (XROWS, W) bf16 with 1e9 pad rows between/around."""
    out = np.full((XROWS, W), PAD, dtype=NP_BF16)
    for i in range(IMGS):
        base = 1 + i * (H + 1)
        out[base : base + H] = shard_bf16[i]
    return out


def kernel(x: np.ndarray, _reps: int = 1):
    assert x.shape == (B, C, H, W)
    xb = _to_bf16(x).reshape(N_CORES, IMGS, H, W)
    nc = _get_nc(_reps)
    in_maps = [{"x": _pad_shard(xb[k])} for k in range(N_CORES)]
    res = run_bass_kernel_spmd(nc, in_maps, core_ids=list(range(N_CORES)))
    out16 = np.stack([r["y"] for r in res.results], axis=0)
    # bf16 -> f32 upcast via bit shift
    out = (out16.view(np.uint16).astype(np.uint32) << 16).view(np.float32)
    return out.reshape(B, C, H, W)


# revision 5
# speedup vs baseline: 1.0731x; 1.0731x over previous
"""Trainium2 Bass kernel for 2D erosion (3x3 sliding-window min) on
x: (8, 4, 1024, 1024) f32, borders padded with +1e9 (pad never wins).

Strategy: pure data parallel over the 32 (b, c) images -> 4 images per core.
Device compute runs in bf16 (harness gate is rel_err < 2e-2; bf16 rounding is
monotone so min commutes with it -> error <= 2^-9): halves DMA bytes and
doubles DVE throughput (2x_1p mode needs 2-byte dtype + unit-stride innermost
dims, which the op formulation below maintains).

Per-core DRAM input is a (4101, 1024) bf16 stack: 4 images with one 1e9 pad
row between/around. Per image, one overlapping load puts DRAM rows
8p-1 .. 8p+8 (10 rows, 20KB) in partition p — the +-1 halo rows ride along,
so the vertical pass needs no cross-partition traffic and no separate halo
DMA.

Vertical (H) pass, 3 DVE ops via pair-sharing (x rows indexed 0..9 in-tile,
v[r] = min over x[r .. r+2]):
    s[k]      = min(x[2k+1], x[2k+2])   k=0..3
    v[even r] = min(x[r], s[r/2])
    v[odd r]  = min(s[(r-1)/2], x[r+2])
All operands are 3D APs with unit-stride 1024-wide innermost dims -> 2x mode.

Horizontal (W) pass, shift formulation (unit strides, 2x mode), with the
final combine offloaded to GPSIMD to balance engine load:
    t[j] = min(v[j], v[j+1])            (DVE)
    o[j] = min(t[j-1], t[j])            (GPSIMD)
Row-boundary columns (first/last of each 1024-wide row) shrink to a 2-tap
window = the adjacent t value; one tiny strided copy per image fixes both
edges of all 8 rows.

Loads ride the SP HWDGE ring, stores the ACT ring, so they don't queue
behind each other.
"""

import numpy as np
import ml_dtypes

import concourse.bass as bass
import concourse.bacc as bacc
import concourse.mybir as mybir
from concourse.tile import TileContext
from concourse.bass_utils import run_bass_kernel_spmd

N_CORES = 8
B, C, H, W = 8, 4, 1024, 1024
IMGS = B * C // N_CORES  # images per core = 4
P = 128                  # SBUF partitions
R = H // P               # image rows per partition = 8
RL = R + 2               # loaded rows per partition (incl +-1 halo)
F = R * W                # free-dim elements per partition = 8192
PAD = 1.0e9
XROWS = IMGS * (H + 1) + 1  # padded per-core input rows = 4101
BF16 = mybir.dt.bfloat16
MIN = mybir.AluOpType.min
NP_BF16 = ml_dtypes.bfloat16

_NC_CACHE = {}


LOOP_BODY_REPS = 4  # reps unrolled inside the hardware loop body


def _emit_image(nc, pools, i):
    """Emit load -> H pass -> W pass -> store for image i."""
    xpool, spool, vpool, tpool, opool = pools
    x, y = nc._x, nc._y
    base = 1 + i * (H + 1)  # first image row in the padded stack

    xt = xpool.tile([P, RL * W], BF16)
    # overlapping load: partition p <- DRAM rows base-1+8p .. base+8+8p
    src = bass.AP(x, (base - 1) * W, [[R * W, P], [1, RL * W]])
    nc.sync.dma_start(out=xt, in_=src)

    # ---- H pass: v[r] = min(x[r], x[r+1], x[r+2]) (tile rows), computed as
    # two flat 1D row-shift mins. On HW only flat unit-stride APs hit the
    # DVE 2x bf16 mode; row-strided 3D APs fall back to 1x.
    a = spool.tile([P, (RL - 1) * W], BF16)  # a[q] = min(x[q], x[q+W])
    v = vpool.tile([P, F], BF16)
    nc.vector.tensor_tensor(
        out=a, in0=xt[:, 0 : (RL - 1) * W], in1=xt[:, W : RL * W], op=MIN
    )
    nc.vector.tensor_tensor(
        out=v, in0=a[:, 0:F], in1=a[:, W : (RL - 1) * W], op=MIN
    )

    # ---- W pass: o[j] = min(v[j-1], v[j], v[j+1]) within rows ----
    t = tpool.tile([P, F], BF16)  # t[0..F-2] valid
    nc.vector.tensor_tensor(
        out=t[:, 0 : F - 1], in0=v[:, 0 : F - 1], in1=v[:, 1:F], op=MIN
    )
    o = opool.tile([P, F], BF16)
    nc.vector.tensor_tensor(
        out=o[:, 1 : F - 1], in0=t[:, 0 : F - 2], in1=t[:, 1 : F - 1], op=MIN
    )
    # per-row first/last column: window shrinks to 2 taps = t value
    orr = o.rearrange("p (r w) -> p r w", r=R)
    tr = t.rearrange("p (r w) -> p r w", r=R)
    nc.vector.tensor_copy(
        out=orr[:, :, 0 : W : W - 1], in_=tr[:, :, 0 : W - 1 : W - 2]
    )

    # store on the ACT HWDGE ring (parallel to SP loads)
    ym = y[i].rearrange("(p r) w -> p (r w)", p=P)
    nc.scalar.dma_start(out=ym, in_=o)


def _build_nc(reps=1):
    nc = bacc.Bacc()
    nc._x = nc.dram_tensor("x", (XROWS, W), BF16, kind="ExternalInput")
    nc._y = nc.dram_tensor("y", (IMGS, H, W), BF16, kind="ExternalOutput")

    with TileContext(nc) as tc:
        with (
            tc.tile_pool(name="xp", bufs=3) as xpool,
            tc.tile_pool(name="sp", bufs=2) as spool,
            tc.tile_pool(name="vp", bufs=2) as vpool,
            tc.tile_pool(name="tp", bufs=2) as tpool,
            tc.tile_pool(name="op", bufs=2) as opool,
        ):
            pools = (xpool, spool, vpool, tpool, opool)
            if reps <= 48:
                for i in [im for _ in range(reps) for im in range(IMGS)]:
                    _emit_image(nc, pools, i)
            else:
                # timing mode: hardware loop keeps the NEFF compact so reps
                # can be large enough to swamp host/tunnel timing noise
                n_iter, rem = divmod(reps, LOOP_BODY_REPS)
                with tc.For_i(0, n_iter, 1):
                    for i in [
                        im for _ in range(LOOP_BODY_REPS) for im in range(IMGS)
                    ]:
                        _emit_image(nc, pools, i)
                for i in [im for _ in range(rem) for im in range(IMGS)]:
                    _emit_image(nc, pools, i)

    nc.finalize()
    return nc


def _get_nc(reps=1):
    if reps not in _NC_CACHE:
        _NC_CACHE[reps] = _build_nc(reps)
    return _NC_CACHE[reps]


def _to_bf16(x):
    """f32 -> bf16 with round-to-nearest-even (vectorized bit trick)."""
    u = np.ascontiguousarray(x, dtype=np.float32).view(np.uint32)
    r = ((u + 0x7FFF + ((u >> 16) & 1)) >> 16).astype(np.uint16)
    return r.view(NP_BF16)


def _pad_shard(shard_bf16):
    """(IMGS, H, W) bf16 -> (XROWS, W) bf16 with 1e9 pad rows between/around."""
    out = np.full((XROWS, W), PAD, dtype=NP_BF16)
    for i in range(IMGS):
        base = 1 + i * (H + 1)
        out[base : base + H] = shard_bf16[i]
    return out


def kernel(x: np.ndarray, _reps: int = 1):
    assert x.shape == (B, C, H, W)
    xb = _to_bf16(x).reshape(N_CORES, IMGS, H, W)
    nc = _get_nc(_reps)
    in_maps = [{"x": _pad_shard(xb[k])} for k in range(N_CORES)]
    res = run_bass_kernel_spmd(nc, in_maps, core_ids=list(range(N_CORES)))
    out16 = np.stack([r["y"] for r in res.results], axis=0)
    # bf16 -> f32 upcast via bit shift
    out = (out16.view(np.uint16).astype(np.uint32) << 16).view(np.float32)
    return out.reshape(B, C, H, W)
